# revision 42
# baseline (speedup 1.0000x reference)
"""BiCGSTAB (4 fixed iterations, 7-point stencil) on 8 Trainium2 NeuronCores.

Problem: x,b,ref: [2,256,256,256] f32, center: [1,256,256,1] f32.
reference() runs 4 BiCGSTAB iterations of A·u where A is the 7-point stencil
  S(u)[b,h,w,z] = center[h,w]*u - u[w-1] - u[w+1] - u[h-1] - u[h+1] - u[z-1] - u[z+1]
with zero Dirichlet boundaries, and global (per-batch) dot products.

Sharding: core c ∈ 0..7 handles batch b=c//4 and H-slab [64*(c%4), 64*(c%4)+64).
Dot products become 4-rank AllReduces in groups [[0..3],[4..7]].
H-halo planes are exchanged via AllGather within the group + indirect-DMA
gathers using per-core row-index tensors (edge cores index a zeroed row range,
implementing the Dirichlet boundary).

On-chip layout: SBUF partition dim = W (2 chunks of 128), free dim = (h, z).
H/Z stencil shifts are free-dim shifted access patterns; W shifts are done on
the TensorEngine as matmuls with a tridiagonal adjacency matrix (plus one-hot
boundary matrices that couple the two W chunks).
"""
import numpy as np

import concourse.bacc as bacc
import concourse.bass as bass
import concourse.mybir as mybir
import concourse.tile as tile

F32 = mybir.dt.float32
I32 = mybir.dt.int32

N_CORES = 8
GROUP = 4  # cores per batch group
EPS = 1e-6


def build_program(HC=64, W=256, Z=256, KH=8, ITERS=4, collectives=True,
                  maxph=99, twin_reps=0):
    """Build the per-core SPMD Bass program. HC = H planes per core.

    collectives=False builds a single-core timing twin (collective_compute
    calls skipped; numerics wrong) usable with TimelineSim.
    """
    assert W % 128 == 0 and W // 128 == 2
    assert HC % KH == 0
    NB = HC // KH  # h blocks per pass
    NCH = (KH * Z + 511) // 512  # psum chunks per out tile
    RG = [list(range(GROUP)), list(range(GROUP, 2 * GROUP))]
    ZR = GROUP * 2 * W  # zero-row base in halo_out

    twin = twin_reps > 0
    assert not (twin and collectives), "twin loop cannot contain collectives"
    nc = bacc.Bacc("TRN2", target_bir_lowering=False, debug=False,
                   num_devices=N_CORES)

    if twin:
        # timing twin: big I/O replaced by internal DRAM; tiny dummy output
        x_in = nc.dram_tensor("xin_t", [HC, W, Z], F32)
        b_in = nc.dram_tensor("bin_t", [HC, W, Z], F32)
        x_out = nc.dram_tensor("xout_t", [HC, W, Z], F32)
        dummy_out = nc.dram_tensor("dummy_o", [1, 8], F32, kind="ExternalOutput")
    else:
        x_in = nc.dram_tensor("x", [HC, W, Z], F32, kind="ExternalInput")
        b_in = nc.dram_tensor("bb", [HC, W, Z], F32, kind="ExternalInput")
        x_out = nc.dram_tensor("xout", [HC, W, Z], F32, kind="ExternalOutput")
    cen_in = nc.dram_tensor("cen", [W, HC], F32, kind="ExternalInput")
    mats_in = nc.dram_tensor("mats", [128, 384], F32, kind="ExternalInput")
    idx_in = nc.dram_tensor("idx", [W, 2], I32, kind="ExternalInput")

    with tile.TileContext(nc) as tc:
        with (
            tc.tile_pool(name="sb", bufs=2) as sb,
            tc.tile_pool(name="ps", bufs=8, space="PSUM") as ps,
            tc.tile_pool(name="dr", bufs=1, space="DRAM") as dr,
        ):
            _cnt = [0]

            def _nm(pfx):
                _cnt[0] += 1
                return f"{pfx}{_cnt[0]}"

            # ---- persistent DRAM intermediates (one tile each, held throughout)
            fld = {n: dr.tile([HC, W, Z], F32, tag=n, name=f"fld_{n}")
                   for n in ("r0", "r", "p", "v", "s", "t", "xw")}
            halo_in = dr.tile([2 * W, Z], F32, tag="halo_in")
            halo_out = dr.tile([ZR + 128, Z], F32, tag="halo_out")
            din = dr.tile([1, 8], F32, tag="din")
            dout = dr.tile([1, 8], F32, tag="dout")

            # ---- persistent SBUF constants
            cen_sb = []
            for wc in range(2):
                c = sb.tile([128, HC], F32, tag=f"cen{wc}", bufs=1)
                nc.sync.dma_start(out=c[:], in_=cen_in[wc * 128:(wc + 1) * 128, :])
                cen_sb.append(c)
            mats_sb = sb.tile([128, 384], F32, tag="mats", bufs=1)
            nc.sync.dma_start(out=mats_sb[:], in_=mats_in[:, :])
            A_ap = mats_sb[:, 0:128]
            B01_ap = mats_sb[:, 128:256]  # adds win1[0] into out0[127]
            B10_ap = mats_sb[:, 256:384]  # adds win0[127] into out1[0]
            idx_sb = []
            for wc in range(2):
                it_ = sb.tile([128, 2], I32, tag=f"idx{wc}", bufs=1)
                nc.sync.dma_start(out=it_[:], in_=idx_in[wc * 128:(wc + 1) * 128, :])
                idx_sb.append(it_)

            # zero the ghost-row tail of halo_out and the unused cols of din
            zt = sb.tile([128, Z], F32, tag="gh")
            nc.vector.memset(zt[:], 0.0)
            nc.sync.dma_start(out=halo_out[ZR:ZR + 128, :], in_=zt[:])
            z8 = sb.tile([1, 8], F32, tag="z8", bufs=1)
            nc.vector.memset(z8[:], 0.0)
            nc.sync.dma_start(out=din[:, :], in_=z8[:])

            # ---- helpers ------------------------------------------------
            def stage_halo_plane(src_sbuf_plane, side, wc):
                """src_sbuf_plane: [128, Z] SBUF AP of boundary plane."""
                r0_ = side * W + wc * 128
                nc.sync.dma_start(out=halo_in[r0_:r0_ + 128, :],
                                  in_=src_sbuf_plane)

            def stage_halo_from_dram(field):
                for wc in range(2):
                    for side, h in ((0, 0), (1, HC - 1)):
                        g = sb.tile([128, Z], F32, tag="gh", name=_nm("gh"))
                        nc.sync.dma_start(
                            out=g[:], in_=field[h, wc * 128:wc * 128 + 128, :])
                        stage_halo_plane(g[:], side, wc)

            def allgather():
                if not collectives:
                    return
                nc.gpsimd.collective_compute(
                    "AllGather", mybir.AluOpType.bypass, replica_groups=RG,
                    ins=[halo_in[:, :].opt()], outs=[halo_out[0:ZR, :].opt()])

            def load_window(field, wc, j, tag):
                """[128, KH+2, Z] window of planes j*KH-1 .. j*KH+KH."""
                h0 = j * KH
                w0 = wc * 128
                win = sb.tile([128, KH + 2, Z], F32, tag=tag, name=_nm("win"))
                lo_g = (j == 0)
                hi_g = (j == NB - 1)
                a = 0 if lo_g else h0 - 1
                bnd = HC if hi_g else h0 + KH + 1
                po = 1 if lo_g else 0
                nc.sync.dma_start(
                    out=win[:, po:po + (bnd - a), :],
                    in_=field[a:bnd, w0:w0 + 128, :].rearrange("h w z -> w h z"))
                if lo_g:
                    nc.gpsimd.indirect_dma_start(
                        out=win[:, 0, :], out_offset=None, in_=halo_out[:, :],
                        in_offset=bass.IndirectOffsetOnAxis(
                            ap=idx_sb[wc][:, 0:1], axis=0))
                if hi_g:
                    nc.gpsimd.indirect_dma_start(
                        out=win[:, KH + 1, :], out_offset=None, in_=halo_out[:, :],
                        in_offset=bass.IndirectOffsetOnAxis(
                            ap=idx_sb[wc][:, 1:2], axis=0))
                return win

            def stencil_tile(wins, wc, j):
                """vt = S(field) for chunk wc, block j. wins = (win0, win1)."""
                h0 = j * KH
                win = wins[wc]
                other = wins[1 - wc]
                t1 = sb.tile([128, KH, Z], F32, tag=f"t1{wc}", name=_nm("t1"))
                nc.vector.tensor_add(out=t1[:], in0=win[:, 0:KH, :],
                                     in1=win[:, 2:KH + 2, :])
                nc.vector.tensor_add(out=t1[:, :, 1:Z], in0=t1[:, :, 1:Z],
                                     in1=win[:, 1:KH + 1, 0:Z - 1])
                nc.vector.tensor_add(out=t1[:, :, 0:Z - 1], in0=t1[:, :, 0:Z - 1],
                                     in1=win[:, 1:KH + 1, 1:Z])
                vt = sb.tile([128, KH, Z], F32, tag=f"vt{wc}", name=_nm("vt"))
                for j1 in range(KH):
                    h = h0 + j1
                    nc.scalar.mul(out=vt[:, j1, :], in_=win[:, j1 + 1, :],
                                  mul=cen_sb[wc][:, h:h + 1])
                nc.vector.tensor_tensor(out=vt[:], in0=vt[:], in1=t1[:],
                                        op=mybir.AluOpType.subtract)
                wf = win[:].rearrange("p h z -> p (h z)")
                of = other[:].rearrange("p h z -> p (h z)")
                vf = vt[:].rearrange("p h z -> p (h z)")
                Bm = B01_ap if wc == 0 else B10_ap
                for q in range(NCH):
                    c0, c1 = q * 512, min((q + 1) * 512, KH * Z)
                    pt = ps.tile([128, c1 - c0], F32, tag="pt", name=_nm("pt"))
                    nc.tensor.matmul(out=pt[:], lhsT=A_ap,
                                     rhs=wf[:, Z + c0:Z + c1],
                                     start=True, stop=False)
                    nc.tensor.matmul(out=pt[:], lhsT=Bm,
                                     rhs=of[:, Z + c0:Z + c1],
                                     start=False, stop=True)
                    nc.vector.tensor_tensor(out=vf[:, c0:c1], in0=vf[:, c0:c1],
                                            in1=pt[:],
                                            op=mybir.AluOpType.subtract)
                return vt, t1

            def store_tile(field, src, wc, j, halo=False):
                h0 = j * KH
                w0 = wc * 128
                nc.sync.dma_start(
                    out=field[h0:h0 + KH, w0:w0 + 128, :].rearrange(
                        "h w z -> w h z"),
                    in_=src[:])
                if halo:
                    if j == 0:
                        stage_halo_plane(src[:, 0, :], 0, wc)
                    if j == NB - 1:
                        stage_halo_plane(src[:, KH - 1, :], 1, wc)

            def ttr(in0, in1, acc_prev, scr, tag="accA"):
                # dot-product partial: scr = in0*in1 (discarded), acc = row sums
                # (tensor_tensor_reduce is avoided: it faults on HW)
                acc = sb.tile([128, 1], F32, tag=tag + "p", bufs=4,
                              name=_nm("acc"))
                nc.vector.scalar_tensor_tensor(
                    out=scr, in0=in0, scalar=1.0, in1=in1,
                    op0=mybir.AluOpType.mult, op1=mybir.AluOpType.mult,
                    accum_out=acc[:])
                if acc_prev is None:
                    return acc
                tot = sb.tile([128, 1], F32, tag=tag, bufs=4, name=_nm("accs"))
                nc.vector.tensor_add(out=tot[:], in0=acc_prev[:], in1=acc[:])
                return tot

            def finish_dot(acc, col):
                dsc = sb.tile([1, 1], F32, tag="dsc", bufs=16, name=_nm("dsc"))
                nc.gpsimd.tensor_reduce(out=dsc[:], in_=acc[:],
                                        axis=mybir.AxisListType.C,
                                        op=mybir.AluOpType.add)
                nc.sync.dma_start(out=din[0:1, col:col + 1], in_=dsc[:])

            def allreduce():
                if collectives:
                    nc.gpsimd.collective_compute(
                        "AllReduce", mybir.AluOpType.add, replica_groups=RG,
                        ins=[din[:, :].opt()], outs=[dout[:, :].opt()])
                dsb = sb.tile([1, 8], F32, tag="dsb", bufs=6, name=_nm("dsb"))
                nc.sync.dma_start(out=dsb[:], in_=dout[:, :])
                return dsb

            def s_tile():
                return sb.tile([1, 1], F32, tag="dsc", bufs=16, name=_nm("sc"))

            def s_recip_eps(a_ap):
                t = s_tile()
                nc.vector.tensor_scalar_add(out=t[:], in0=a_ap, scalar1=EPS)
                r_ = s_tile()
                nc.vector.reciprocal(out=r_[:], in_=t[:])
                return r_

            def s_mul(a_ap, b_ap):
                t = s_tile()
                nc.vector.tensor_tensor(out=t[:], in0=a_ap, in1=b_ap,
                                        op=mybir.AluOpType.mult)
                return t

            def s_neg(a_ap):
                t = s_tile()
                nc.vector.tensor_scalar_mul(out=t[:], in0=a_ap, scalar1=-1.0)
                return t

            def bcast(a_ap):
                b_ = sb.tile([128, 1], F32, tag="bc", bufs=8, name=_nm("bc"))
                nc.gpsimd.partition_broadcast(b_[:], a_ap, channels=128)
                return b_

            def stt(out, in0, sc, in1):
                """out = in0*sc + in1 (sc: [128,1] AP)."""
                nc.vector.scalar_tensor_tensor(
                    out=out, in0=in0, scalar=sc, in1=in1,
                    op0=mybir.AluOpType.mult, op1=mybir.AluOpType.add)

            def load_blk(field, wc, j, tag):
                t_ = sb.tile([128, KH, Z], F32, tag=tag, name=_nm("blk"))
                h0 = j * KH
                w0 = wc * 128
                nc.sync.dma_start(
                    out=t_[:],
                    in_=field[h0:h0 + KH, w0:w0 + 128, :].rearrange(
                        "h w z -> w h z"))
                return t_

            # block order: interior blocks first so ghost-dependent blocks can
            # overlap with the AllGather still in flight.
            border = [j for j in range(NB) if 0 < j < NB - 1]
            border += [0] if NB == 1 else [0, NB - 1]

            # ================= P0: r0 = b - S(x); rho = <r0,r0> ===========
            from contextlib import ExitStack as _ES
            _loop = _ES()
            if twin:
                _loop.enter_context(tc.For_i(0, twin_reps, 1))
            stage_halo_from_dram(x_in)
            allgather()
            acc = None
            rho_ap = None
            if maxph >= 2:
                for j in border:
                    wins = (load_window(x_in, 0, j, "win0"),
                            load_window(x_in, 1, j, "win1"))
                    for wc in range(2):
                        vt, t1 = stencil_tile(wins, wc, j)
                        bt = load_blk(b_in, wc, j, "lA")
                        r0t = sb.tile([128, KH, Z], F32, tag=f"o{wc}",
                                      name=_nm("r0t"))
                        nc.vector.tensor_tensor(out=r0t[:], in0=bt[:],
                                                in1=vt[:],
                                                op=mybir.AluOpType.subtract)
                        acc = ttr(r0t[:], r0t[:], acc, t1[:])
                        store_tile(fld["r0"], r0t, wc, j, halo=True)
                finish_dot(acc, 0)
                allreduce_out = allreduce()
                rho_ap = allreduce_out[0:1, 0:1]
                allgather()  # r0 boundary planes = p/r ghosts for iteration 0

            for it in range(ITERS if maxph >= 3 else 0):
                last = (it == ITERS - 1)
                p_src = fld["r0"] if it == 0 else fld["p"]
                r_src = fld["r0"] if it == 0 else fld["r"]
                x_src = x_in if it == 0 else fld["xw"]
                x_dst = x_out if last else fld["xw"]

                # ===== P1: v = S(p); d1 = <r0, v> =====
                acc = None
                for j in border:
                    wins = (load_window(p_src, 0, j, "win0"),
                            load_window(p_src, 1, j, "win1"))
                    for wc in range(2):
                        vt, t1 = stencil_tile(wins, wc, j)
                        if it == 0:
                            # p == r0: the window centre planes ARE r0
                            r0_ap = wins[wc][:, 1:KH + 1, :]
                        else:
                            r0_ap = load_blk(fld["r0"], wc, j, "lA")[:]
                        acc = ttr(r0_ap, vt[:], acc, t1[:])
                        store_tile(fld["v"], vt, wc, j)
                finish_dot(acc, 0)
                dsb = allreduce()
                alpha = s_mul(rho_ap, s_recip_eps(dsb[0:1, 0:1])[:])
                alpha_bc = bcast(alpha[:])
                nalpha_bc = bcast(s_neg(alpha[:])[:])
                if maxph < 4:
                    break

                # ===== P2: s = r - alpha*v =====
                # halo-producing blocks first so the AllGather overlaps the rest
                ew_order = ([0, NB - 1] if NB > 1 else [0]) + list(range(1, NB - 1))
                for wc in range(2):
                    for j in ew_order:
                        rt = load_blk(r_src, wc, j, "lA")
                        vt_ = load_blk(fld["v"], wc, j, "lB")
                        st = sb.tile([128, KH, Z], F32, tag=f"o{wc}",
                                     name=_nm("st"))
                        stt(st[:], vt_[:], nalpha_bc[:], rt[:])
                        store_tile(fld["s"], st, wc, j, halo=True)
                allgather()
                if maxph < 5:
                    break

                # ===== P3: t = S(s); <t,s>, <t,t> =====
                accA = accB = None
                for j in border:
                    wins = (load_window(fld["s"], 0, j, "win0"),
                            load_window(fld["s"], 1, j, "win1"))
                    for wc in range(2):
                        vt, t1 = stencil_tile(wins, wc, j)
                        accA = ttr(wins[wc][:, 1:KH + 1, :], vt[:], accA,
                                   t1[:], "accA")
                        accB = ttr(vt[:], vt[:], accB, t1[:], "accB")
                        if not last:
                            store_tile(fld["t"], vt, wc, j)
                finish_dot(accA, 0)
                finish_dot(accB, 1)
                dsb = allreduce()
                omega = s_mul(dsb[0:1, 0:1], s_recip_eps(dsb[0:1, 1:2])[:])
                omega_bc = bcast(omega[:])
                nomega_bc = bcast(s_neg(omega[:])[:])
                if maxph < 6:
                    break

                # ===== P4: x += alpha*p + omega*s; r = s - omega*t =====
                acc = None
                for wc in range(2):
                    for j in range(NB):
                        xt = load_blk(x_src, wc, j, "lA")
                        pt_ = load_blk(p_src, wc, j, "lB")
                        st = load_blk(fld["s"], wc, j, "t10")
                        x1 = sb.tile([128, KH, Z], F32, tag="scrB",
                                     name=_nm("x1"))
                        stt(x1[:], pt_[:], alpha_bc[:], xt[:])
                        x2 = sb.tile([128, KH, Z], F32, tag=f"o{wc}",
                                     name=_nm("x2"))
                        stt(x2[:], st[:], omega_bc[:], x1[:])
                        store_tile(x_dst, x2, wc, j)
                        if not last:
                            tt = load_blk(fld["t"], wc, j, "win0")
                            r0t = load_blk(fld["r0"], wc, j, "win1")
                            rt = sb.tile([128, KH, Z], F32, tag="t11",
                                         name=_nm("rt"))
                            stt(rt[:], tt[:], nomega_bc[:], st[:])
                            acc = ttr(r0t[:], rt[:], acc, x1[:])
                            store_tile(fld["r"], rt, wc, j)
                if last:
                    break
                finish_dot(acc, 0)
                dsb = allreduce()
                beta = s_mul(s_mul(dsb[0:1, 0:1], s_recip_eps(rho_ap)[:])[:],
                             s_mul(alpha[:], s_recip_eps(omega[:])[:])[:])
                beta_bc = bcast(beta[:])
                rho_ap = dsb[0:1, 0:1]

                # ===== P5: p = r + beta*(p - omega*v) =====
                for wc in range(2):
                    for j in ew_order:
                        rt = load_blk(fld["r"], wc, j, "lA")
                        pt_ = load_blk(p_src, wc, j, "lB")
                        vt_ = load_blk(fld["v"], wc, j, "t10")
                        u = sb.tile([128, KH, Z], F32, tag="scrB",
                                    name=_nm("u"))
                        stt(u[:], vt_[:], nomega_bc[:], pt_[:])
                        po = sb.tile([128, KH, Z], F32, tag=f"o{wc}",
                                     name=_nm("po"))
                        stt(po[:], u[:], beta_bc[:], rt[:])
                        store_tile(fld["p"], po, wc, j, halo=True)
                allgather()

            _loop.close()
            if twin:
                nc.sync.dma_start(out=dummy_out[:, :], in_=z8[:])

    nc.compile()
    return nc


# ---------------------------------------------------------------------------
# host-side wrapper
# ---------------------------------------------------------------------------
_CACHE = {}


def _shift_mats():
    A = np.zeros((128, 128), np.float32)
    for i in range(127):
        A[i, i + 1] = 1.0
        A[i + 1, i] = 1.0
    B01 = np.zeros((128, 128), np.float32)
    B01[0, 127] = 1.0
    B10 = np.zeros((128, 128), np.float32)
    B10[127, 0] = 1.0
    return np.concatenate([A, B01, B10], axis=1)


def make_in_maps(x, b, center, HC, W, Z):
    """Slice full inputs into per-core input maps."""
    mats = _shift_mats()
    ZR = GROUP * 2 * W
    in_maps = []
    for c in range(N_CORES):
        bi, s = divmod(c, GROUP)
        h0 = s * HC
        cen = center[0, h0:h0 + HC, :, 0].astype(np.float32).T.copy()  # [W, HC]
        w = np.arange(W, dtype=np.int32)
        lo = (s - 1) * 2 * W + W + w if s > 0 else ZR + (w % 128)
        hi = (s + 1) * 2 * W + w if s < GROUP - 1 else ZR + (w % 128)
        idx = np.stack([lo, hi], axis=1).astype(np.int32)
        in_maps.append({
            "x": np.ascontiguousarray(x[bi, h0:h0 + HC]),
            "bb": np.ascontiguousarray(b[bi, h0:h0 + HC]),
            "cen": cen,
            "mats": mats,
            "idx": idx,
        })
    return in_maps


RUN_WALL_S = []  # wall-clock of each device dispatch (incl. axon h2d/d2h)


def kernel(x, b, ref, center):
    """Full inputs in, full output out. ref is unused by the reference model."""
    import time as _time
    B, H, W, Z = x.shape
    HC = H // GROUP
    key = (HC, W, Z)
    if key not in _CACHE:
        _CACHE[key] = build_program(HC=HC, W=W, Z=Z)
    nc = _CACHE[key]

    from concourse.bass_utils import run_bass_kernel_spmd
    in_maps = make_in_maps(np.asarray(x), np.asarray(b), np.asarray(center),
                           HC, W, Z)
    _t0 = _time.time()
    res = run_bass_kernel_spmd(nc, in_maps, core_ids=list(range(N_CORES)))
    RUN_WALL_S.append(_time.time() - _t0)
    out = np.empty((B, H, W, Z), np.float32)
    for c in range(N_CORES):
        bi, s = divmod(c, GROUP)
        out[bi, s * HC:(s + 1) * HC] = res.results[c]["xout"]
    return out


# revision 48
# speedup vs baseline: 1242.4294x; 1242.4294x over previous
"""BiCGSTAB (4 fixed iterations, 7-point stencil) on 8 Trainium2 NeuronCores.

Problem: x,b,ref: [2,256,256,256] f32, center: [1,256,256,1] f32.
reference() runs 4 BiCGSTAB iterations of A·u where A is the 7-point stencil
  S(u)[b,h,w,z] = center[h,w]*u - u[w-1] - u[w+1] - u[h-1] - u[h+1] - u[z-1] - u[z+1]
with zero Dirichlet boundaries, and global (per-batch) dot products.

Sharding: core c ∈ 0..7 handles batch b=c//4 and H-slab [64*(c%4), 64*(c%4)+64).
Dot products become 4-rank AllReduces in groups [[0..3],[4..7]].
H-halo planes are exchanged via AllGather within the group + indirect-DMA
gathers using per-core row-index tensors (edge cores index a zeroed row range,
implementing the Dirichlet boundary).

On-chip layout: SBUF partition dim = W (2 chunks of 128), free dim = (h, z).
H/Z stencil shifts are free-dim shifted access patterns; W shifts are done on
the TensorEngine as matmuls with a tridiagonal adjacency matrix (plus one-hot
boundary matrices that couple the two W chunks).
"""
import numpy as np

import concourse.bacc as bacc
import concourse.bass as bass
import concourse.mybir as mybir
import concourse.tile as tile

F32 = mybir.dt.float32
I32 = mybir.dt.int32

N_CORES = 8
GROUP = 4  # cores per batch group
EPS = 1e-6


def build_program(HC=64, W=256, Z=256, KH=8, ITERS=4, collectives=True,
                  maxph=99, twin_reps=0):
    """Build the per-core SPMD Bass program. HC = H planes per core.

    collectives=False builds a single-core timing twin (collective_compute
    calls skipped; numerics wrong) usable with TimelineSim.
    """
    assert W % 128 == 0 and W // 128 == 2
    assert HC % KH == 0
    NB = HC // KH  # h blocks per pass
    NCH = (KH * Z + 511) // 512  # psum chunks per out tile
    RG = [list(range(GROUP)), list(range(GROUP, 2 * GROUP))]
    ZR = GROUP * 2 * W  # zero-row base in halo_out

    twin = twin_reps > 0
    assert not (twin and collectives), "twin loop cannot contain collectives"
    nc = bacc.Bacc("TRN2", target_bir_lowering=False, debug=False,
                   num_devices=N_CORES)

    if twin:
        # timing twin: big I/O replaced by internal DRAM; tiny dummy output
        x_in = nc.dram_tensor("xin_t", [HC, W, Z], F32)
        b_in = nc.dram_tensor("bin_t", [HC, W, Z], F32)
        x_out = nc.dram_tensor("xout_t", [HC, W, Z], F32)
        dummy_out = nc.dram_tensor("dummy_o", [1, 8], F32, kind="ExternalOutput")
    else:
        x_in = nc.dram_tensor("x", [HC, W, Z], F32, kind="ExternalInput")
        b_in = nc.dram_tensor("bb", [HC, W, Z], F32, kind="ExternalInput")
        x_out = nc.dram_tensor("xout", [HC, W, Z], F32, kind="ExternalOutput")
    cen_in = nc.dram_tensor("cen", [W, HC], F32, kind="ExternalInput")
    mats_in = nc.dram_tensor("mats", [128, 384], F32, kind="ExternalInput")
    idx_in = nc.dram_tensor("idx", [W, 2], I32, kind="ExternalInput")

    with tile.TileContext(nc) as tc:
        with (
            tc.tile_pool(name="sb", bufs=2) as sb,
            tc.tile_pool(name="ps", bufs=8, space="PSUM") as ps,
            tc.tile_pool(name="dr", bufs=1, space="DRAM") as dr,
        ):
            _cnt = [0]

            def _nm(pfx):
                _cnt[0] += 1
                return f"{pfx}{_cnt[0]}"

            # ---- persistent DRAM intermediates (one tile each, held throughout)
            fld = {n: dr.tile([HC, W, Z], F32, tag=n, name=f"fld_{n}")
                   for n in ("r0", "r", "p", "v", "s", "t", "xw")}
            halo_in = dr.tile([2 * W, Z], F32, tag="halo_in")
            halo_out = dr.tile([ZR + 128, Z], F32, tag="halo_out")
            din = dr.tile([1, 8], F32, tag="din")
            dout = dr.tile([1, 8], F32, tag="dout")

            # ---- persistent SBUF constants
            cen_sb = []
            for wc in range(2):
                c = sb.tile([128, HC], F32, tag=f"cen{wc}", bufs=1)
                nc.sync.dma_start(out=c[:], in_=cen_in[wc * 128:(wc + 1) * 128, :])
                cen_sb.append(c)
            mats_sb = sb.tile([128, 384], F32, tag="mats", bufs=1)
            nc.sync.dma_start(out=mats_sb[:], in_=mats_in[:, :])
            A_ap = mats_sb[:, 0:128]
            B01_ap = mats_sb[:, 128:256]  # adds win1[0] into out0[127]
            B10_ap = mats_sb[:, 256:384]  # adds win0[127] into out1[0]
            idx_sb = []
            for wc in range(2):
                it_ = sb.tile([128, 2], I32, tag=f"idx{wc}", bufs=1)
                nc.sync.dma_start(out=it_[:], in_=idx_in[wc * 128:(wc + 1) * 128, :])
                idx_sb.append(it_)

            # zero the ghost-row tail of halo_out and the unused cols of din
            zt = sb.tile([128, Z], F32, tag="gh")
            nc.vector.memset(zt[:], 0.0)
            nc.sync.dma_start(out=halo_out[ZR:ZR + 128, :], in_=zt[:])
            z8 = sb.tile([1, 8], F32, tag="z8", bufs=1)
            nc.vector.memset(z8[:], 0.0)
            nc.sync.dma_start(out=din[:, :], in_=z8[:])

            # ---- helpers ------------------------------------------------
            def stage_halo_plane(src_sbuf_plane, side, wc):
                """src_sbuf_plane: [128, Z] SBUF AP of boundary plane."""
                r0_ = side * W + wc * 128
                nc.sync.dma_start(out=halo_in[r0_:r0_ + 128, :],
                                  in_=src_sbuf_plane)

            def stage_halo_from_dram(field):
                for wc in range(2):
                    for side, h in ((0, 0), (1, HC - 1)):
                        g = sb.tile([128, Z], F32, tag="gh", name=_nm("gh"))
                        nc.sync.dma_start(
                            out=g[:], in_=field[h, wc * 128:wc * 128 + 128, :])
                        stage_halo_plane(g[:], side, wc)

            def allgather():
                if not collectives:
                    return
                nc.gpsimd.collective_compute(
                    "AllGather", mybir.AluOpType.bypass, replica_groups=RG,
                    ins=[halo_in[:, :].opt()], outs=[halo_out[0:ZR, :].opt()])

            def load_window(field, wc, j, tag):
                """[128, KH+2, Z] window of planes j*KH-1 .. j*KH+KH."""
                h0 = j * KH
                w0 = wc * 128
                win = sb.tile([128, KH + 2, Z], F32, tag=tag, name=_nm("win"))
                lo_g = (j == 0)
                hi_g = (j == NB - 1)
                a = 0 if lo_g else h0 - 1
                bnd = HC if hi_g else h0 + KH + 1
                po = 1 if lo_g else 0
                nc.sync.dma_start(
                    out=win[:, po:po + (bnd - a), :],
                    in_=field[a:bnd, w0:w0 + 128, :].rearrange("h w z -> w h z"))
                if lo_g:
                    nc.gpsimd.indirect_dma_start(
                        out=win[:, 0, :], out_offset=None, in_=halo_out[:, :],
                        in_offset=bass.IndirectOffsetOnAxis(
                            ap=idx_sb[wc][:, 0:1], axis=0))
                if hi_g:
                    nc.gpsimd.indirect_dma_start(
                        out=win[:, KH + 1, :], out_offset=None, in_=halo_out[:, :],
                        in_offset=bass.IndirectOffsetOnAxis(
                            ap=idx_sb[wc][:, 1:2], axis=0))
                return win

            def stencil_tile(wins, wc, j):
                """vt = S(field) for chunk wc, block j. wins = (win0, win1)."""
                h0 = j * KH
                win = wins[wc]
                other = wins[1 - wc]
                t1 = sb.tile([128, KH, Z], F32, tag=f"t1{wc}", name=_nm("t1"))
                nc.vector.tensor_add(out=t1[:], in0=win[:, 0:KH, :],
                                     in1=win[:, 2:KH + 2, :])
                nc.vector.tensor_add(out=t1[:, :, 1:Z], in0=t1[:, :, 1:Z],
                                     in1=win[:, 1:KH + 1, 0:Z - 1])
                nc.vector.tensor_add(out=t1[:, :, 0:Z - 1], in0=t1[:, :, 0:Z - 1],
                                     in1=win[:, 1:KH + 1, 1:Z])
                vt = sb.tile([128, KH, Z], F32, tag=f"vt{wc}", name=_nm("vt"))
                for j1 in range(KH):
                    h = h0 + j1
                    nc.scalar.mul(out=vt[:, j1, :], in_=win[:, j1 + 1, :],
                                  mul=cen_sb[wc][:, h:h + 1])
                nc.vector.tensor_tensor(out=vt[:], in0=vt[:], in1=t1[:],
                                        op=mybir.AluOpType.subtract)
                wf = win[:].rearrange("p h z -> p (h z)")
                of = other[:].rearrange("p h z -> p (h z)")
                vf = vt[:].rearrange("p h z -> p (h z)")
                Bm = B01_ap if wc == 0 else B10_ap
                for q in range(NCH):
                    c0, c1 = q * 512, min((q + 1) * 512, KH * Z)
                    pt = ps.tile([128, c1 - c0], F32, tag="pt", name=_nm("pt"))
                    nc.tensor.matmul(out=pt[:], lhsT=A_ap,
                                     rhs=wf[:, Z + c0:Z + c1],
                                     start=True, stop=False)
                    nc.tensor.matmul(out=pt[:], lhsT=Bm,
                                     rhs=of[:, Z + c0:Z + c1],
                                     start=False, stop=True)
                    nc.vector.tensor_tensor(out=vf[:, c0:c1], in0=vf[:, c0:c1],
                                            in1=pt[:],
                                            op=mybir.AluOpType.subtract)
                return vt, t1

            def store_tile(field, src, wc, j, halo=False):
                h0 = j * KH
                w0 = wc * 128
                nc.sync.dma_start(
                    out=field[h0:h0 + KH, w0:w0 + 128, :].rearrange(
                        "h w z -> w h z"),
                    in_=src[:])
                if halo:
                    if j == 0:
                        stage_halo_plane(src[:, 0, :], 0, wc)
                    if j == NB - 1:
                        stage_halo_plane(src[:, KH - 1, :], 1, wc)

            def ttr(in0, in1, acc_prev, scr, tag="accA"):
                # dot-product partial: scr = in0*in1 (discarded), acc = row sums
                # (tensor_tensor_reduce is avoided: it faults on HW)
                acc = sb.tile([128, 1], F32, tag=tag + "p", bufs=4,
                              name=_nm("acc"))
                nc.vector.scalar_tensor_tensor(
                    out=scr, in0=in0, scalar=1.0, in1=in1,
                    op0=mybir.AluOpType.mult, op1=mybir.AluOpType.mult,
                    accum_out=acc[:])
                if acc_prev is None:
                    return acc
                tot = sb.tile([128, 1], F32, tag=tag, bufs=4, name=_nm("accs"))
                nc.vector.tensor_add(out=tot[:], in0=acc_prev[:], in1=acc[:])
                return tot

            def finish_dot(acc, col):
                dsc = sb.tile([1, 1], F32, tag="dsc", bufs=16, name=_nm("dsc"))
                nc.gpsimd.tensor_reduce(out=dsc[:], in_=acc[:],
                                        axis=mybir.AxisListType.C,
                                        op=mybir.AluOpType.add)
                nc.sync.dma_start(out=din[0:1, col:col + 1], in_=dsc[:])

            def allreduce():
                if collectives:
                    nc.gpsimd.collective_compute(
                        "AllReduce", mybir.AluOpType.add, replica_groups=RG,
                        ins=[din[:, :].opt()], outs=[dout[:, :].opt()])
                dsb = sb.tile([1, 8], F32, tag="dsb", bufs=6, name=_nm("dsb"))
                nc.sync.dma_start(out=dsb[:], in_=dout[:, :])
                return dsb

            def s_tile():
                return sb.tile([1, 1], F32, tag="dsc", bufs=16, name=_nm("sc"))

            def s_recip_eps(a_ap):
                t = s_tile()
                nc.vector.tensor_scalar_add(out=t[:], in0=a_ap, scalar1=EPS)
                r_ = s_tile()
                nc.vector.reciprocal(out=r_[:], in_=t[:])
                return r_

            def s_mul(a_ap, b_ap):
                t = s_tile()
                nc.vector.tensor_tensor(out=t[:], in0=a_ap, in1=b_ap,
                                        op=mybir.AluOpType.mult)
                return t

            def s_sub(a_ap, b_ap):
                t = s_tile()
                nc.vector.tensor_tensor(out=t[:], in0=a_ap, in1=b_ap,
                                        op=mybir.AluOpType.subtract)
                return t

            def s_neg(a_ap):
                t = s_tile()
                nc.vector.tensor_scalar_mul(out=t[:], in0=a_ap, scalar1=-1.0)
                return t

            def bcast(a_ap):
                b_ = sb.tile([128, 1], F32, tag="bc", bufs=8, name=_nm("bc"))
                nc.gpsimd.partition_broadcast(b_[:], a_ap, channels=128)
                return b_

            def stt(out, in0, sc, in1):
                """out = in0*sc + in1 (sc: [128,1] AP)."""
                nc.vector.scalar_tensor_tensor(
                    out=out, in0=in0, scalar=sc, in1=in1,
                    op0=mybir.AluOpType.mult, op1=mybir.AluOpType.add)

            def load_blk(field, wc, j, tag):
                t_ = sb.tile([128, KH, Z], F32, tag=tag, name=_nm("blk"))
                h0 = j * KH
                w0 = wc * 128
                nc.sync.dma_start(
                    out=t_[:],
                    in_=field[h0:h0 + KH, w0:w0 + 128, :].rearrange(
                        "h w z -> w h z"))
                return t_

            # block order: interior blocks first so ghost-dependent blocks can
            # overlap with the AllGather still in flight.
            border = [j for j in range(NB) if 0 < j < NB - 1]
            border += [0] if NB == 1 else [0, NB - 1]

            # ================= P0: r0 = b - S(x); rho = <r0,r0> ===========
            from contextlib import ExitStack as _ES
            _loop = _ES()
            if twin:
                _loop.enter_context(tc.For_i(0, twin_reps, 1))
            stage_halo_from_dram(x_in)
            allgather()
            acc = None
            rho_ap = None
            if maxph >= 2:
                for j in border:
                    wins = (load_window(x_in, 0, j, "win0"),
                            load_window(x_in, 1, j, "win1"))
                    for wc in range(2):
                        vt, t1 = stencil_tile(wins, wc, j)
                        bt = load_blk(b_in, wc, j, "lA")
                        r0t = sb.tile([128, KH, Z], F32, tag=f"o{wc}",
                                      name=_nm("r0t"))
                        nc.vector.tensor_tensor(out=r0t[:], in0=bt[:],
                                                in1=vt[:],
                                                op=mybir.AluOpType.subtract)
                        acc = ttr(r0t[:], r0t[:], acc, t1[:])
                        store_tile(fld["r0"], r0t, wc, j, halo=True)
                finish_dot(acc, 0)
                allreduce_out = allreduce()
                rho_ap = allreduce_out[0:1, 0:1]
                allgather()  # r0 boundary planes = p/r ghosts for iteration 0

            for it in range(ITERS if maxph >= 3 else 0):
                last = (it == ITERS - 1)
                p_src = fld["r0"] if it == 0 else fld["p"]
                r_src = fld["r0"] if it == 0 else fld["r"]
                x_src = x_in if it == 0 else fld["xw"]
                x_dst = x_out if last else fld["xw"]

                # ===== P1: v = S(p); d1 = <r0, v> =====
                acc = None
                for j in border:
                    wins = (load_window(p_src, 0, j, "win0"),
                            load_window(p_src, 1, j, "win1"))
                    for wc in range(2):
                        vt, t1 = stencil_tile(wins, wc, j)
                        if it == 0:
                            # p == r0: the window centre planes ARE r0
                            r0_ap = wins[wc][:, 1:KH + 1, :]
                        else:
                            r0_ap = load_blk(fld["r0"], wc, j, "lA")[:]
                        acc = ttr(r0_ap, vt[:], acc, t1[:])
                        store_tile(fld["v"], vt, wc, j)
                finish_dot(acc, 0)
                dsb = allreduce()
                d1_ap = dsb[0:1, 0:1]
                alpha = s_mul(rho_ap, s_recip_eps(d1_ap)[:])
                alpha_bc = bcast(alpha[:])
                nalpha_bc = bcast(s_neg(alpha[:])[:])
                if maxph < 4:
                    break

                # ===== P2: s = r - alpha*v =====
                # halo-producing blocks first so the AllGather overlaps the rest
                ew_order = ([0, NB - 1] if NB > 1 else [0]) + list(range(1, NB - 1))
                for wc in range(2):
                    for j in ew_order:
                        rt = load_blk(r_src, wc, j, "lA")
                        vt_ = load_blk(fld["v"], wc, j, "lB")
                        st = sb.tile([128, KH, Z], F32, tag=f"o{wc}",
                                     name=_nm("st"))
                        stt(st[:], vt_[:], nalpha_bc[:], rt[:])
                        store_tile(fld["s"], st, wc, j, halo=True)
                allgather()
                if maxph < 5:
                    break

                # ===== P3: t = S(s); <t,s>, <t,t>, <r0,t> =====
                accA = accB = accC = None
                for j in border:
                    wins = (load_window(fld["s"], 0, j, "win0"),
                            load_window(fld["s"], 1, j, "win1"))
                    for wc in range(2):
                        vt, t1 = stencil_tile(wins, wc, j)
                        accA = ttr(wins[wc][:, 1:KH + 1, :], vt[:], accA,
                                   t1[:], "accA")
                        accB = ttr(vt[:], vt[:], accB, t1[:], "accB")
                        if not last:
                            r0t = load_blk(fld["r0"], wc, j, "lA")
                            accC = ttr(r0t[:], vt[:], accC, t1[:], "accC")
                            store_tile(fld["t"], vt, wc, j)
                finish_dot(accA, 0)
                finish_dot(accB, 1)
                if not last:
                    finish_dot(accC, 2)
                dsb = allreduce()
                omega = s_mul(dsb[0:1, 0:1], s_recip_eps(dsb[0:1, 1:2])[:])
                omega_bc = bcast(omega[:])
                nomega_bc = bcast(s_neg(omega[:])[:])
                if not last:
                    # rho' = <r0, s - w*t> = (rho - alpha*d1) - omega*<r0,t>
                    rho_n = s_sub(s_sub(rho_ap, s_mul(alpha[:], d1_ap)[:])[:],
                                  s_mul(omega[:], dsb[0:1, 2:3])[:])
                    beta = s_mul(
                        s_mul(rho_n[:], s_recip_eps(rho_ap)[:])[:],
                        s_mul(alpha[:], s_recip_eps(omega[:])[:])[:])
                    beta_bc = bcast(beta[:])
                    rho_ap = rho_n[:]
                if maxph < 6:
                    break

                # ===== P4+P5 fused: x += alpha*p + omega*s;
                #       r = s - omega*t;  p = r + beta*(p - omega*v) =====
                for wc in range(2):
                    for j in (ew_order if not last else list(range(NB))):
                        xt = load_blk(x_src, wc, j, "lA")
                        pt_ = load_blk(p_src, wc, j, "lB")
                        st = load_blk(fld["s"], wc, j, "t10")
                        x1 = sb.tile([128, KH, Z], F32, tag="scrB",
                                     name=_nm("x1"))
                        stt(x1[:], pt_[:], alpha_bc[:], xt[:])
                        x2 = sb.tile([128, KH, Z], F32, tag=f"o{wc}",
                                     name=_nm("x2"))
                        stt(x2[:], st[:], omega_bc[:], x1[:])
                        store_tile(x_dst, x2, wc, j)
                        if not last:
                            tt = load_blk(fld["t"], wc, j, "win0")
                            vt_ = load_blk(fld["v"], wc, j, "win1")
                            rt = sb.tile([128, KH, Z], F32, tag="t11",
                                         name=_nm("rt"))
                            stt(rt[:], tt[:], nomega_bc[:], st[:])
                            store_tile(fld["r"], rt, wc, j)
                            u = sb.tile([128, KH, Z], F32, tag="scrB",
                                        name=_nm("u"))
                            stt(u[:], vt_[:], nomega_bc[:], pt_[:])
                            po = sb.tile([128, KH, Z], F32, tag="uB",
                                         name=_nm("po"))
                            stt(po[:], u[:], beta_bc[:], rt[:])
                            store_tile(fld["p"], po, wc, j, halo=True)
                if last:
                    break
                allgather()

            _loop.close()
            if twin:
                nc.sync.dma_start(out=dummy_out[:, :], in_=z8[:])

    nc.compile()
    return nc


# ---------------------------------------------------------------------------
# host-side wrapper
# ---------------------------------------------------------------------------
_CACHE = {}


def _shift_mats():
    A = np.zeros((128, 128), np.float32)
    for i in range(127):
        A[i, i + 1] = 1.0
        A[i + 1, i] = 1.0
    B01 = np.zeros((128, 128), np.float32)
    B01[0, 127] = 1.0
    B10 = np.zeros((128, 128), np.float32)
    B10[127, 0] = 1.0
    return np.concatenate([A, B01, B10], axis=1)


def make_in_maps(x, b, center, HC, W, Z):
    """Slice full inputs into per-core input maps."""
    mats = _shift_mats()
    ZR = GROUP * 2 * W
    in_maps = []
    for c in range(N_CORES):
        bi, s = divmod(c, GROUP)
        h0 = s * HC
        cen = center[0, h0:h0 + HC, :, 0].astype(np.float32).T.copy()  # [W, HC]
        w = np.arange(W, dtype=np.int32)
        lo = (s - 1) * 2 * W + W + w if s > 0 else ZR + (w % 128)
        hi = (s + 1) * 2 * W + w if s < GROUP - 1 else ZR + (w % 128)
        idx = np.stack([lo, hi], axis=1).astype(np.int32)
        in_maps.append({
            "x": np.ascontiguousarray(x[bi, h0:h0 + HC]),
            "bb": np.ascontiguousarray(b[bi, h0:h0 + HC]),
            "cen": cen,
            "mats": mats,
            "idx": idx,
        })
    return in_maps


RUN_WALL_S = []  # wall-clock of each device dispatch (incl. axon h2d/d2h)


def kernel(x, b, ref, center):
    """Full inputs in, full output out. ref is unused by the reference model."""
    import time as _time
    B, H, W, Z = x.shape
    HC = H // GROUP
    key = (HC, W, Z)
    if key not in _CACHE:
        _CACHE[key] = build_program(HC=HC, W=W, Z=Z)
    nc = _CACHE[key]

    from concourse.bass_utils import run_bass_kernel_spmd
    in_maps = make_in_maps(np.asarray(x), np.asarray(b), np.asarray(center),
                           HC, W, Z)
    _t0 = _time.time()
    res = run_bass_kernel_spmd(nc, in_maps, core_ids=list(range(N_CORES)))
    RUN_WALL_S.append(_time.time() - _t0)
    out = np.empty((B, H, W, Z), np.float32)
    for c in range(N_CORES):
        bi, s = divmod(c, GROUP)
        out[bi, s * HC:(s + 1) * HC] = res.results[c]["xout"]
    return out
